# revision 13
# baseline (speedup 1.0000x reference)
"""NeuronMemory retrieval-KNN kernel for 8 Trainium2 NeuronCores.

Reference computation (per token t with D=1024, R=64, NC=16, NK=16384, K=8):
  Q[t,r]    = sum_n sum_d mw[t,n] * x[t,d] * cn[n,d,r]
  scores    = Q @ kK.T / sqrt(R)                       [t, NK]
  topk_scores, topk_idx = top_k(scores, K)
  w         = softmax(topk_scores)
  out[t,:]  = sum_k w[t,k] * kV[topk_idx[t,k], :]

Sharding: data-parallel over the 8192 flattened (B,S) tokens -> 1024
tokens per core; compress pool / knowledge tensors replicated.

Per-core pipeline (8 token-tiles of 128):
  phase B:  Y = X @ W_all via PE (fp32), Q = sum_n mw_n*Y_n on DVE,
            Q.T via PE transpose.
  phase C:  scores in 2 halves of 8192 on PE -> PSUM -> SBUF (ACT),
            top-8 via DVE max / max_index (exact fp32, jax tie order),
            softmax (ACT exp + DVE), V-row gather via SWDGE dma_gather,
            weighted combine as diag(w) matmuls accumulated in PSUM.
"""

import numpy as np

import concourse.bacc as bacc
import concourse.mybir as mybir
from concourse.masks import make_identity
from concourse.tile import TileContext
from concourse.bass_utils import run_bass_kernel_spmd

# problem constants
D = 1024
RANK = 64
NCMP = 16
NK = 16384
KNN = 8
B, S = 2, 4096
NCORES = 8
T = (B * S) // NCORES          # tokens per core = 1024
TT = T // 128                  # token tiles per core = 8
DC = D // 128                  # d chunks = 8
NH = 2                         # knowledge halves
HK = NK // NH                  # 8192 per half
HCC = HK // 1024               # 1024-wide psum chunks per half = 8

F32 = mybir.dt.float32
F32R = mybir.dt.float32r

# matmul dtypes: fp32 is exact (4 cyc/row), fp32r is fast (1 cyc/row) but
# looser on HW and requires an explicitly fp32r-rounded dataflow.
Q_MM_DT = F32
S_MM_DT = F32
C_MM_DT = F32


def _mm_cast(ap, dt):
    return ap.bitcast(dt) if dt != F32 else ap


def build_nc():
    nc = bacc.Bacc("TRN2", target_bir_lowering=False)

    xT = nc.declare_dram_parameter("xT", [TT, 128, DC * 128], F32, isOutput=False)
    wall = nc.declare_dram_parameter("wall", [128, DC * 1024], F32, isOutput=False)
    mw = nc.declare_dram_parameter("mw", [128, TT * NCMP], F32, isOutput=False)
    # kt packed [128, 8192]: rows 0-63 = K.T/8 for kn 0..8191, rows 64-127
    # for kn 8192..16383 -> the two halves run as concurrent PE row groups.
    kt = nc.declare_dram_parameter("kt", [128, HK], F32, isOutput=False)
    vmat = nc.declare_dram_parameter("v", [NK, D], F32, isOutput=False)

    out = nc.declare_dram_parameter("out", [T, D], F32, isOutput=True)
    idxo = nc.declare_dram_parameter("idx", [T, KNN], mybir.dt.int32, isOutput=True)
    w8o = nc.declare_dram_parameter("w8", [T, KNN], F32, isOutput=True)

    idxstage = nc.dram_tensor("idxstage", [T * KNN], mybir.dt.int16)

    with TileContext(nc) as tc:
        with tc.tile_pool(name="persist", bufs=1) as pp:
            kt_sb = pp.tile([128, HK], F32)
            wall_sb = pp.tile([128, DC * 1024], F32)
            mw_sb = pp.tile([128, TT * NCMP], F32)
            ident = pp.tile([128, 128], F32)
            # rows 0-63: Q.T; rows 64-127: duplicate (for row group 1)
            qt_sb = pp.tile([128, T], F32)
            idxw = pp.tile([128, TT * (1024 // 16)], mybir.dt.int16)

            nc.sync.dma_start(out=kt_sb[:], in_=kt[:])
            nc.sync.dma_start(out=wall_sb[:], in_=wall[:])
            nc.sync.dma_start(out=mw_sb[:], in_=mw[:])
            make_identity(nc, ident[:])

            # ---------------- phase B: Q then Q.T ----------------
            with tc.tile_pool(name="phaseB", bufs=2) as pb, \
                 tc.tile_pool(name="phaseB_ps", bufs=1, space="PSUM") as pbp:
                for tt in range(TT):
                    xt = pb.tile([128, DC * 128], F32, tag="xt")
                    nc.sync.dma_start(out=xt[:], in_=xT[tt])
                    # contraction split into two concurrent PE row groups
                    psya = pbp.tile([128, NCMP * RANK], F32, tag="psya")
                    psyb = pbp.tile([128, NCMP * RANK], F32, tag="psyb")
                    for dc in range(DC):
                        for lo, psy in ((0, psya), (64, psyb)):
                            lhs = _mm_cast(
                                xt[lo:lo + 64, dc * 128:(dc + 1) * 128],
                                Q_MM_DT)
                            for half in range(2):
                                nc.tensor.matmul(
                                    psy[:, half * 512:(half + 1) * 512],
                                    lhsT=lhs,
                                    rhs=_mm_cast(
                                        wall_sb[lo:lo + 64,
                                                dc * 1024 + half * 512:
                                                dc * 1024 + (half + 1) * 512],
                                        Q_MM_DT),
                                    start=(dc == 0),
                                    stop=(dc == DC - 1),
                                )
                    ysa = pb.tile([128, NCMP * RANK], F32, tag="ysa")
                    nc.scalar.activation(
                        ysa[:], psya[:], mybir.ActivationFunctionType.Copy)
                    ysum = pb.tile([128, NCMP * RANK], F32, tag="ysum")
                    nc.vector.tensor_add(ysum[:], ysa[:], psyb[:])
                    # m = Y * mw  (broadcast mw over r)
                    m = pb.tile([128, NCMP * RANK], F32, tag="m")
                    mw_b = (mw_sb[:, tt * NCMP:(tt + 1) * NCMP]
                            .to_broadcast([128, NCMP, RANK]))
                    nc.vector.tensor_tensor(
                        out=m[:].rearrange("p (n r) -> p n r", n=NCMP),
                        in0=ysum[:].rearrange("p (n r) -> p n r", n=NCMP),
                        in1=mw_b,
                        op=mybir.AluOpType.mult,
                    )
                    # fold 16 -> 1 over n
                    a1 = pb.tile([128, 512], F32, tag="a1")
                    nc.vector.tensor_add(a1[:], m[:, 0:512], m[:, 512:1024])
                    a2 = pb.tile([128, 256], F32, tag="a2")
                    nc.vector.tensor_add(a2[:], a1[:, 0:256], a1[:, 256:512])
                    a3 = pb.tile([128, 128], F32, tag="a3")
                    nc.vector.tensor_add(a3[:], a2[:, 0:128], a2[:, 128:256])
                    qtile = pb.tile([128, RANK], F32, tag="qtile")
                    nc.vector.tensor_add(qtile[:], a3[:, 0:64], a3[:, 64:128])
                    # Q.T via PE transpose; duplicate into rows 64-127
                    psqt = pbp.tile([RANK, 128], F32, tag="psqt")
                    nc.tensor.transpose(psqt[:], qtile[:], ident[:])
                    nc.scalar.activation(
                        qt_sb[0:RANK, tt * 128:(tt + 1) * 128], psqt[:],
                        mybir.ActivationFunctionType.Copy)
                    nc.sync.dma_start(
                        out=qt_sb[64:64 + RANK, tt * 128:(tt + 1) * 128],
                        in_=qt_sb[0:RANK, tt * 128:(tt + 1) * 128])

            # ---------------- phase C ----------------
            with tc.tile_pool(name="scores", bufs=1) as scp, \
                 tc.tile_pool(name="small", bufs=2) as smp, \
                 tc.tile_pool(name="vg", bufs=1) as vgp, \
                 tc.tile_pool(name="diag", bufs=2) as dgp, \
                 tc.tile_pool(name="outp", bufs=2) as otp, \
                 tc.tile_pool(name="sc_ps", bufs=2, space="PSUM") as scps, \
                 tc.tile_pool(name="o_ps", bufs=1, space="PSUM") as ops:
                for tt in range(TT):
                    sc0 = scp.tile([128, HK], F32, tag="sc0")
                    sc1 = scp.tile([128, HK], F32, tag="sc1")
                    cand = smp.tile([128, 2 * KNN], F32, tag="cand")
                    for cc in range(HK // 512):
                        ps = scps.tile([128, 1024], F32, tag="ps")
                        cs = slice(cc * 512, (cc + 1) * 512)
                        for g, (lo, sc) in enumerate(((0, sc0), (64, sc1))):
                            nc.tensor.matmul(
                                ps[:, g * 512:(g + 1) * 512],
                                lhsT=_mm_cast(
                                    qt_sb[lo:lo + RANK,
                                          tt * 128:(tt + 1) * 128],
                                    S_MM_DT),
                                rhs=_mm_cast(kt_sb[lo:lo + RANK, cs], S_MM_DT),
                                start=True, stop=True,
                                tile_position=(lo, 0),
                            )
                        for g, sc in enumerate((sc0, sc1)):
                            nc.scalar.activation(
                                sc[:, cs], ps[:, g * 512:(g + 1) * 512],
                                mybir.ActivationFunctionType.Copy)
                    for h, sc in ((0, sc0), (1, sc1)):
                        nc.vector.max(out=cand[:, h * KNN:(h + 1) * KNN], in_=sc[:])
                    top8 = smp.tile([128, KNN], F32, tag="top8")
                    nc.vector.max(out=top8[:], in_=cand[:])
                    i0 = smp.tile([128, KNN], mybir.dt.uint32, tag="i0")
                    i1 = smp.tile([128, KNN], mybir.dt.uint32, tag="i1")
                    nc.vector.max_index(i0[:], top8[:], sc0[:])
                    nc.vector.max_index(i1[:], top8[:], sc1[:])
                    i0f = smp.tile([128, KNN], F32, tag="i0f")
                    i1f = smp.tile([128, KNN], F32, tag="i1f")
                    nc.vector.tensor_copy(i0f[:], i0[:])
                    nc.vector.tensor_copy(i1f[:], i1[:])
                    nc.vector.tensor_scalar(
                        i1f[:], i1f[:], float(HK), scalar2=None,
                        op0=mybir.AluOpType.add)
                    idxf = smp.tile([128, KNN], F32, tag="idxf")
                    nc.vector.tensor_tensor(
                        out=idxf[:], in0=i0f[:], in1=i1f[:],
                        op=mybir.AluOpType.min)
                    idx32 = smp.tile([128, KNN], mybir.dt.int32, tag="idx32")
                    nc.vector.tensor_copy(idx32[:], idxf[:])
                    nc.sync.dma_start(
                        out=idxo[tt * 128:(tt + 1) * 128, :], in_=idx32[:])
                    # softmax over top8 (exp unnormalized + sum via ACT accum)
                    negmax = smp.tile([128, 1], F32, tag="negmax")
                    nc.vector.tensor_scalar(
                        negmax[:], top8[:, 0:1], -1.0, scalar2=None,
                        op0=mybir.AluOpType.mult)
                    e8 = smp.tile([128, KNN], F32, tag="e8")
                    s8 = smp.tile([128, 1], F32, tag="s8")
                    nc.scalar.activation(
                        e8[:], top8[:], mybir.ActivationFunctionType.Exp,
                        bias=negmax[:], scale=1.0, accum_out=s8[:])
                    rcp = smp.tile([128, 1], F32, tag="rcp")
                    nc.vector.reciprocal(rcp[:], s8[:])
                    w8t = smp.tile([128, KNN], F32, tag="w8t")
                    nc.vector.tensor_scalar(
                        w8t[:], e8[:], rcp[:], scalar2=None,
                        op0=mybir.AluOpType.mult)
                    nc.sync.dma_start(
                        out=w8o[tt * 128:(tt + 1) * 128, :], in_=w8t[:])
                    # idx -> int16, bounce through DRAM into gather layout
                    idx16 = smp.tile([128, KNN], mybir.dt.int16, tag="idx16")
                    nc.vector.tensor_copy(idx16[:], idxf[:])
                    stage_t = idxstage[:].rearrange(
                        "(a k t) -> a t k", a=TT, k=KNN, t=128)[tt]
                    nc.sync.dma_start(out=stage_t, in_=idx16[:])
                    wrap_src = idxstage[:].rearrange(
                        "(a f p) -> a p f", a=TT, f=64, p=16)[tt]
                    for g in range(8):
                        nc.sync.dma_start(
                            out=idxw[16 * g:16 * (g + 1),
                                     tt * 64:(tt + 1) * 64],
                            in_=wrap_src)
                    # gather selected V rows: out[t, k, :] = V[idx[t, k]]
                    vg = vgp.tile([128, KNN, D], F32, tag="vg")
                    nc.gpsimd.dma_gather(
                        out_ap=vg[:],
                        in_ap=vmat[:],
                        idxs_ap=idxw[:, tt * 64:(tt + 1) * 64],
                        num_idxs=KNN * 128,
                        num_idxs_reg=KNN * 128,
                        elem_size=D,
                    )
                    # weighted combine: psum += diag(e8_k) @ vg_k, split into
                    # two concurrent 64x64 diagonal-block row/col groups
                    pso = ops.tile([128, 1024], F32, tag="pso")
                    for k in range(KNN):
                        dg = dgp.tile([128, 128], F32, tag="dg")
                        nc.scalar.activation(
                            dg[:], ident[:], mybir.ActivationFunctionType.Copy,
                            scale=e8[:, k:k + 1])
                        for half in range(2):
                            for lo in (0, 64):
                                nc.tensor.matmul(
                                    pso[lo:lo + 64,
                                        half * 512:(half + 1) * 512],
                                    lhsT=_mm_cast(
                                        dg[lo:lo + 64, lo:lo + 64], C_MM_DT),
                                    rhs=_mm_cast(
                                        vg[lo:lo + 64, k,
                                           half * 512:(half + 1) * 512],
                                        C_MM_DT),
                                    start=(k == 0), stop=(k == KNN - 1),
                                    tile_position=(lo, lo),
                                )
                    outsb = otp.tile([128, 1024], F32, tag="outsb")
                    nc.scalar.activation(
                        outsb[:], pso[:], mybir.ActivationFunctionType.Copy,
                        scale=rcp[:])
                    nc.sync.dma_start(
                        out=out[tt * 128:(tt + 1) * 128, :], in_=outsb[:])

    nc.compile()
    return nc


_NC_CACHE = None


def _get_nc():
    global _NC_CACHE
    if _NC_CACHE is None:
        _NC_CACHE = build_nc()
    return _NC_CACHE


def prep_core_inputs(x, memory_weights, compress_neurons, knowledge_K,
                     knowledge_V):
    """Full inputs -> list of 8 per-core input dicts."""
    xf = np.ascontiguousarray(x, dtype=np.float32).reshape(B * S, D)
    mwf = np.ascontiguousarray(memory_weights, dtype=np.float32).reshape(
        B * S, NCMP)
    cn = np.ascontiguousarray(compress_neurons, dtype=np.float32)
    kK = np.ascontiguousarray(knowledge_K, dtype=np.float32)
    kV = np.ascontiguousarray(knowledge_V, dtype=np.float32)

    wall = np.ascontiguousarray(
        cn.transpose(1, 0, 2).reshape(DC, 128, NCMP * RANK)
        .transpose(1, 0, 2).reshape(128, DC * NCMP * RANK))
    ktT = kK.T / np.float32(np.sqrt(RANK))          # [64, NK]
    kt = np.ascontiguousarray(
        np.concatenate([ktT[:, :HK], ktT[:, HK:]], axis=0))  # [128, HK]

    in_maps = []
    for c in range(NCORES):
        xs = xf[c * T:(c + 1) * T]
        x_prep = np.ascontiguousarray(
            xs.reshape(TT, 128, DC, 128).transpose(0, 3, 2, 1)
            .reshape(TT, 128, DC * 128))
        mws = mwf[c * T:(c + 1) * T]
        mw_prep = np.ascontiguousarray(
            mws.reshape(TT, 128, NCMP).transpose(1, 0, 2)
            .reshape(128, TT * NCMP))
        in_maps.append({
            "xT": x_prep,
            "wall": wall,
            "mw": mw_prep,
            "kt": kt,
            "v": kV,
        })
    return in_maps


def run_on_cores(inputs, trace=False, **kw):
    nc = _get_nc()
    in_maps = prep_core_inputs(**inputs)
    res = run_bass_kernel_spmd(
        nc, in_maps, core_ids=list(range(NCORES)), trace=trace, **kw)
    outs = np.concatenate(
        [res.results[c]["out"] for c in range(NCORES)], axis=0)
    idxs = np.concatenate(
        [res.results[c]["idx"] for c in range(NCORES)], axis=0)
    w8s = np.concatenate(
        [res.results[c]["w8"] for c in range(NCORES)], axis=0)
    output = outs.reshape(B, S, D)
    topk_idx = idxs.reshape(B, S, KNN).astype(np.int32)
    weights = w8s.reshape(B, S, KNN)
    return (output, topk_idx, weights), res


def kernel(**inputs):
    (output, topk_idx, weights), _ = run_on_cores(inputs, trace=False)
    return output, topk_idx, weights


# revision 21
# speedup vs baseline: 1.2668x; 1.2668x over previous
"""NeuronMemory retrieval-KNN kernel for 8 Trainium2 NeuronCores.

Reference computation (per token t with D=1024, R=64, NC=16, NK=16384, K=8):
  Q[t,r]    = sum_n sum_d mw[t,n] * x[t,d] * cn[n,d,r]
  scores    = Q @ kK.T / sqrt(R)                       [t, NK]
  topk_scores, topk_idx = top_k(scores, K)
  w         = softmax(topk_scores)
  out[t,:]  = sum_k w[t,k] * kV[topk_idx[t,k], :]

Sharding: data-parallel over the 8192 flattened (B,S) tokens -> 1024
tokens per core; compress pool / knowledge tensors replicated.

Per-core pipeline (8 token-tiles of 128):
  phase B:  Y = X @ W_all via PE, Q = sum_n mw_n*Y_n on DVE,
            Q.T via PE transpose (duplicated to both partition halves).
  phase C:  scores into 4 quarter-buffers of 4096 (6-deep pool for
            cross-tile pipelining); kt is packed [128, 8192] so each
            PSUM chunk computes two knowledge halves as PE row groups.
            top-8 via DVE max / max_index per quarter + index merge
            (exact fp32, jax tie order), softmax (ACT exp + DVE),
            V-row gather via SWDGE dma_gather, weighted combine as
            diag(w) matmuls accumulated in PSUM.
"""

import numpy as np

import concourse.bacc as bacc
import concourse.mybir as mybir
from concourse.masks import make_identity
from concourse.tile import TileContext
from concourse.bass_utils import run_bass_kernel_spmd

# problem constants
D = 1024
RANK = 64
NCMP = 16
NK = 16384
KNN = 8
B, S = 2, 4096
NCORES = 8
T = (B * S) // NCORES          # tokens per core = 1024
TT = T // 128                  # token tiles per core = 8
DC = D // 128                  # d chunks = 8
HK = NK // 2                   # kn per packed-kt row group = 8192
NQ = 4                         # scan quarters
QK = NK // NQ                  # 4096 per quarter

F32 = mybir.dt.float32
F32R = mybir.dt.float32r

# matmul dtypes: fp32 is exact (2-pass on PE), fp32r is faster but rounds
# inputs on HW — measured 246/65536 top-k index flips, so Q/scores MUST be
# fp32. The V-combine only feeds the 2e-2-tolerance output -> fp32r OK.
Q_MM_DT = F32
S_MM_DT = F32
C_MM_DT = F32R


def build_nc():
    nc = bacc.Bacc("TRN2", target_bir_lowering=False)

    xT = nc.declare_dram_parameter("xT", [TT, 128, DC * 128], Q_MM_DT,
                                   isOutput=False)
    wall = nc.declare_dram_parameter("wall", [128, DC * 1024], Q_MM_DT,
                                     isOutput=False)
    mw = nc.declare_dram_parameter("mw", [128, TT * NCMP], F32, isOutput=False)
    # kt packed [128, 8192]: rows 0-63 = K.T/8 for kn 0..8191, rows 64-127
    # for kn 8192..16383 -> two knowledge halves per PSUM chunk.
    kt = nc.declare_dram_parameter("kt", [128, HK], S_MM_DT, isOutput=False)
    vmat = nc.declare_dram_parameter("v", [NK, D], C_MM_DT, isOutput=False)

    out = nc.declare_dram_parameter("out", [T, D], F32, isOutput=True)
    idxo = nc.declare_dram_parameter("idx", [T, KNN], mybir.dt.int32,
                                     isOutput=True)
    w8o = nc.declare_dram_parameter("w8", [T, KNN], F32, isOutput=True)

    idxstage = nc.dram_tensor("idxstage", [T * KNN], mybir.dt.int16)

    with TileContext(nc) as tc:
        with tc.tile_pool(name="persist", bufs=1) as pp:
            kt_sb = pp.tile([128, HK], S_MM_DT)
            mw_sb = pp.tile([128, TT * NCMP], F32)
            ident = pp.tile([128, 128], F32)
            # rows 0-63: Q.T; rows 64-127: duplicate (for row group 1)
            qt_sb = pp.tile([128, T], S_MM_DT)
            idxw = pp.tile([128, TT * (1024 // 16)], mybir.dt.int16)

            nc.sync.dma_start(out=kt_sb[:], in_=kt[:])
            nc.sync.dma_start(out=mw_sb[:], in_=mw[:])
            make_identity(nc, ident[:])

            # ---------------- phase B: Q then Q.T ----------------
            with tc.tile_pool(name="phaseB_w", bufs=1) as pbw, \
                 tc.tile_pool(name="phaseB", bufs=2) as pb, \
                 tc.tile_pool(name="phaseB_ps", bufs=2, space="PSUM") as pbp:
                wall_sb = pbw.tile([128, DC * 1024], Q_MM_DT)
                nc.sync.dma_start(out=wall_sb[:], in_=wall[:])
                for tt in range(TT):
                    xt = pb.tile([128, DC * 128], Q_MM_DT, tag="xt")
                    nc.sync.dma_start(out=xt[:], in_=xT[tt])
                    psy = pbp.tile([128, NCMP * RANK], F32, tag="psy")
                    for dc in range(DC):
                        lhs = xt[:, dc * 128:(dc + 1) * 128]
                        for half in range(2):
                            nc.tensor.matmul(
                                psy[:, half * 512:(half + 1) * 512],
                                lhsT=lhs,
                                rhs=wall_sb[:, dc * 1024 + half * 512:
                                            dc * 1024 + (half + 1) * 512],
                                start=(dc == 0),
                                stop=(dc == DC - 1),
                            )
                    # m = Y * mw  (broadcast mw over r)
                    m = pb.tile([128, NCMP * RANK], F32, tag="m")
                    mw_b = (mw_sb[:, tt * NCMP:(tt + 1) * NCMP]
                            .to_broadcast([128, NCMP, RANK]))
                    nc.vector.tensor_tensor(
                        out=m[:].rearrange("p (n r) -> p n r", n=NCMP),
                        in0=psy[:].rearrange("p (n r) -> p n r", n=NCMP),
                        in1=mw_b,
                        op=mybir.AluOpType.mult,
                    )
                    # fold 16 -> 1 over n
                    a1 = pb.tile([128, 512], F32, tag="a1")
                    nc.vector.tensor_add(a1[:], m[:, 0:512], m[:, 512:1024])
                    a2 = pb.tile([128, 256], F32, tag="a2")
                    nc.vector.tensor_add(a2[:], a1[:, 0:256], a1[:, 256:512])
                    a3 = pb.tile([128, 128], F32, tag="a3")
                    nc.vector.tensor_add(a3[:], a2[:, 0:128], a2[:, 128:256])
                    qtile = pb.tile([128, RANK], F32, tag="qtile")
                    nc.vector.tensor_add(qtile[:], a3[:, 0:64], a3[:, 64:128])
                    # Q.T via PE transpose; duplicate into rows 64-127
                    psqt = pbp.tile([RANK, 128], F32, tag="psqt")
                    nc.tensor.transpose(psqt[:], qtile[:], ident[:])
                    nc.scalar.activation(
                        qt_sb[0:RANK, tt * 128:(tt + 1) * 128], psqt[:],
                        mybir.ActivationFunctionType.Copy)
                    nc.sync.dma_start(
                        out=qt_sb[64:64 + RANK, tt * 128:(tt + 1) * 128],
                        in_=qt_sb[0:RANK, tt * 128:(tt + 1) * 128])

            # ---------------- phase C ----------------
            with tc.tile_pool(name="scores", bufs=2) as scp, \
                 tc.tile_pool(name="small", bufs=2) as smp, \
                 tc.tile_pool(name="vg", bufs=1) as vgp, \
                 tc.tile_pool(name="diag", bufs=2) as dgp, \
                 tc.tile_pool(name="outp", bufs=1) as otp, \
                 tc.tile_pool(name="sc_ps", bufs=3, space="PSUM") as scps, \
                 tc.tile_pool(name="o_ps", bufs=1, space="PSUM") as ops:
                for tt in range(TT):
                    # full score row per token tile, double-buffered for
                    # cross-tile overlap. kn halves: cols 0..8191 from kt
                    # rows 0-63, cols 8192.. from rows 64-127.
                    scf = scp.tile([128, NK], F32, tag="scf")
                    cand = smp.tile([128, NQ * KNN], F32, tag="cand")
                    for g, lo in ((0, 0), (1, 64)):
                        for cc in range(HK // 1024):
                            ps = scps.tile([128, 1024], F32, tag="ps")
                            for h in range(2):
                                cs = slice(cc * 1024 + h * 512,
                                           cc * 1024 + (h + 1) * 512)
                                nc.tensor.matmul(
                                    ps[:, h * 512:(h + 1) * 512],
                                    lhsT=qt_sb[lo:lo + RANK,
                                               tt * 128:(tt + 1) * 128],
                                    rhs=kt_sb[lo:lo + RANK, cs],
                                    start=True, stop=True,
                                    tile_position=(lo, 0),
                                )
                            nc.scalar.activation(
                                scf[:, g * HK + cc * 1024:
                                    g * HK + (cc + 1) * 1024],
                                ps[:],
                                mybir.ActivationFunctionType.Copy)
                        # quarter max8s behind production
                        for q in (2 * g, 2 * g + 1):
                            nc.vector.max(
                                out=cand[:, q * KNN:(q + 1) * KNN],
                                in_=scf[:, q * QK:(q + 1) * QK])
                    top8 = smp.tile([128, KNN], F32, tag="top8")
                    nc.vector.max(out=top8[:], in_=cand[:])
                    # single full-row index find (huge fixed cost, tiny
                    # marginal cost -> one call beats per-quarter calls)
                    iq = smp.tile([128, KNN], mybir.dt.uint32, tag="iq")
                    nc.vector.max_index(iq[:], top8[:], scf[:])
                    idx32 = smp.tile([128, KNN], mybir.dt.int32, tag="idx32")
                    nc.vector.tensor_copy(idx32[:], iq[:])
                    nc.scalar.dma_start(
                        out=idxo[tt * 128:(tt + 1) * 128, :], in_=idx32[:])
                    # softmax over top8 (unnormalized exp + sum via ACT accum)
                    negmax = smp.tile([128, 1], F32, tag="negmax")
                    nc.vector.tensor_scalar(
                        negmax[:], top8[:, 0:1], -1.0, scalar2=None,
                        op0=mybir.AluOpType.mult)
                    e8 = smp.tile([128, KNN], F32, tag="e8")
                    s8 = smp.tile([128, 1], F32, tag="s8")
                    nc.scalar.activation(
                        e8[:], top8[:], mybir.ActivationFunctionType.Exp,
                        bias=negmax[:], scale=1.0, accum_out=s8[:])
                    rcp = smp.tile([128, 1], F32, tag="rcp")
                    nc.vector.reciprocal(rcp[:], s8[:])
                    w8t = smp.tile([128, KNN], F32, tag="w8t")
                    nc.vector.tensor_scalar(
                        w8t[:], e8[:], rcp[:], scalar2=None,
                        op0=mybir.AluOpType.mult)
                    nc.scalar.dma_start(
                        out=w8o[tt * 128:(tt + 1) * 128, :], in_=w8t[:])
                    # idx -> int16, bounce through DRAM into gather layout
                    idx16 = smp.tile([128, KNN], mybir.dt.int16, tag="idx16")
                    nc.vector.tensor_copy(idx16[:], iq[:])
                    stage_t = idxstage[:].rearrange(
                        "(a k t) -> a t k", a=TT, k=KNN, t=128)[tt]
                    nc.sync.dma_start(out=stage_t, in_=idx16[:])
                    wrap_src = idxstage[:].rearrange(
                        "(a f p) -> a p f", a=TT, f=64, p=16)[tt]
                    for g in range(8):
                        nc.sync.dma_start(
                            out=idxw[16 * g:16 * (g + 1),
                                     tt * 64:(tt + 1) * 64],
                            in_=wrap_src)
                    # gather selected V rows: out[t, k, :] = V[idx[t, k]]
                    vg = vgp.tile([128, KNN, D], C_MM_DT, tag="vg")
                    nc.gpsimd.dma_gather(
                        out_ap=vg[:],
                        in_ap=vmat[:],
                        idxs_ap=idxw[:, tt * 64:(tt + 1) * 64],
                        num_idxs=KNN * 128,
                        num_idxs_reg=KNN * 128,
                        elem_size=D,
                    )
                    # weighted combine: psum += diag(e8_k) @ vg_k
                    pso = ops.tile([128, 1024], F32, tag="pso")
                    for k in range(KNN):
                        dg = dgp.tile([128, 128], C_MM_DT, tag="dg")
                        nc.gpsimd.tensor_scalar(
                            dg[:], ident[:], e8[:, k:k + 1], scalar2=None,
                            op0=mybir.AluOpType.mult)
                        for half in range(2):
                            nc.tensor.matmul(
                                pso[:, half * 512:(half + 1) * 512],
                                lhsT=dg[:],
                                rhs=vg[:, k, half * 512:(half + 1) * 512],
                                start=(k == 0), stop=(k == KNN - 1),
                            )
                    outsb = otp.tile([128, 1024], F32, tag="outsb")
                    nc.scalar.activation(
                        outsb[:], pso[:], mybir.ActivationFunctionType.Copy,
                        scale=rcp[:])
                    nc.scalar.dma_start(
                        out=out[tt * 128:(tt + 1) * 128, :], in_=outsb[:])

    nc.compile()
    return nc


_NC_CACHE = None


def _get_nc():
    global _NC_CACHE
    if _NC_CACHE is None:
        _NC_CACHE = build_nc()
    return _NC_CACHE


def prep_core_inputs(x, memory_weights, compress_neurons, knowledge_K,
                     knowledge_V):
    """Full inputs -> list of 8 per-core input dicts."""
    xf = np.ascontiguousarray(x, dtype=np.float32).reshape(B * S, D)
    mwf = np.ascontiguousarray(memory_weights, dtype=np.float32).reshape(
        B * S, NCMP)
    cn = np.ascontiguousarray(compress_neurons, dtype=np.float32)
    kK = np.ascontiguousarray(knowledge_K, dtype=np.float32)
    kV = np.ascontiguousarray(knowledge_V, dtype=np.float32)

    wall = np.ascontiguousarray(
        cn.transpose(1, 0, 2).reshape(DC, 128, NCMP * RANK)
        .transpose(1, 0, 2).reshape(128, DC * NCMP * RANK))
    ktT = kK.T / np.float32(np.sqrt(RANK))          # [64, NK]
    kt = np.ascontiguousarray(
        np.concatenate([ktT[:, :HK], ktT[:, HK:]], axis=0))  # [128, HK]

    in_maps = []
    for c in range(NCORES):
        xs = xf[c * T:(c + 1) * T]
        x_prep = np.ascontiguousarray(
            xs.reshape(TT, 128, DC, 128).transpose(0, 3, 2, 1)
            .reshape(TT, 128, DC * 128))
        mws = mwf[c * T:(c + 1) * T]
        mw_prep = np.ascontiguousarray(
            mws.reshape(TT, 128, NCMP).transpose(1, 0, 2)
            .reshape(128, TT * NCMP))
        in_maps.append({
            "xT": x_prep,
            "wall": wall,
            "mw": mw_prep,
            "kt": kt,
            "v": kV,
        })
    return in_maps


def run_on_cores(inputs, trace=False, **kw):
    nc = _get_nc()
    in_maps = prep_core_inputs(**inputs)
    res = run_bass_kernel_spmd(
        nc, in_maps, core_ids=list(range(NCORES)), trace=trace, **kw)
    outs = np.concatenate(
        [res.results[c]["out"] for c in range(NCORES)], axis=0)
    idxs = np.concatenate(
        [res.results[c]["idx"] for c in range(NCORES)], axis=0)
    w8s = np.concatenate(
        [res.results[c]["w8"] for c in range(NCORES)], axis=0)
    output = outs.reshape(B, S, D)
    topk_idx = idxs.reshape(B, S, KNN).astype(np.int32)
    weights = w8s.reshape(B, S, KNN)
    return (output, topk_idx, weights), res


def kernel(**inputs):
    (output, topk_idx, weights), _ = run_on_cores(inputs, trace=False)
    return output, topk_idx, weights
